# revision 13
# baseline (speedup 1.0000x reference)

# MoE BitNet FFN (E=16, D=1024, F=4096, top-1) on 8 Trainium2 NeuronCores.
#
# Strategy: expert-parallel, 2 experts per core. Routing/top-k/dispatch
# bookkeeping runs on host with the exact same jnp ops as the reference
# (bitwise-matching argmax decisions). The heavy FFN runs on device as
# bf16 matmuls over integer-valued quantized operands, which is EXACT:
#   xi  = clip(round(x*s), -128, 127)            ints in [-127,127] (bf16-exact)
#   w1t = clip(round(w1/scale1), -1, 1)          ternary (bf16-exact)
#   Hint = xi @ w1t                              ints |.| <= 2^17  (fp32 PSUM exact)
#   G    = relu(Hint)^2 ; gq = RNE(G*127/maxG)   ints in [0,127]
#   Oint = gq @ w2t                              ints |.| <= 2^19  (exact)
#   out  = Oint * (tp * scale2 * (scale1/s)^2 / 127) * maxG
# All data-dependent scale factors except maxG are host-precomputed per row.

import sys
for p in ("/opt/trn_rl_repo", "/root/.axon_site/_ro/trn_rl_repo"):
    if p not in sys.path:
        sys.path.append(p)

import numpy as np
import ml_dtypes

D_MODEL = 1024
D_FF = 4096
N_EXPERTS = 16
N_CORES = 8
EXP_PER_CORE = 2
MAGIC = 12582912.0  # 1.5 * 2^23, fp32 RNE magic
F32d = None  # set lazily
_prog_cache = {}


def _build_program(T_E):
    import concourse.mybir as mybir
    import concourse.tile as tile
    from concourse.tile_rust import add_dep_helper
    from concourse import bacc

    F32 = mybir.dt.float32
    BF16 = mybir.dt.bfloat16
    CAP = T_E * 128
    NROWS = EXP_PER_CORE * CAP
    NT = EXP_PER_CORE * T_E          # token tiles per core
    KC1 = D_MODEL // 128             # 8  k-chunks layer1
    NJ1 = D_FF // 512                # 8  n-chunks layer1
    KC2 = D_FF // 128                # 32 k-chunks layer2
    NJ2 = D_MODEL // 512             # 2  n-chunks layer2

    nc = bacc.Bacc(None, target_bir_lowering=False)
    xit_d = nc.dram_tensor("xit", (NT, 128, KC1 * 128), BF16, kind="ExternalInput")
    w1_d = nc.dram_tensor("w1c", (EXP_PER_CORE, 128, KC1 * D_FF), BF16, kind="ExternalInput")
    w2_d = nc.dram_tensor("w2c", (EXP_PER_CORE, 128, KC2 * D_MODEL), BF16, kind="ExternalInput")
    cv_d = nc.dram_tensor("cvec", (128, NT), F32, kind="ExternalInput")
    out_d = nc.dram_tensor("outc", (NROWS, D_MODEL), F32, kind="ExternalOutput")

    with tile.TileContext(nc) as tc:
        with (
            tc.tile_pool(name="wts", bufs=1) as wp,
            tc.tile_pool(name="sb", bufs=2) as sb,
            tc.tile_pool(name="xip", bufs=2) as xip,
            tc.tile_pool(name="gqp", bufs=1) as gqp,
            tc.tile_pool(name="scal", bufs=2) as scp,
            tc.tile_pool(name="ps1", bufs=3, space="PSUM") as ps1,
            tc.tile_pool(name="ps2", bufs=2, space="PSUM") as ps2,
        ):
            cvec = scp.tile([128, NT], F32, tag="cvec", bufs=1)
            nc.scalar.dma_start(cvec[:], cv_d[:])

            wh = {}  # slot -> {"w1": (w1a, w1b), "w2": tile}

            KH = KC1 // 2

            def load_w1(slot):
                # halves stream in parallel on two otherwise-idle DMA queues
                w1a = wp.tile([128, KH, D_FF], BF16, tag="w1a")
                w1b = wp.tile([128, KH, D_FF], BF16, tag="w1b")
                ia = nc.scalar.dma_start(w1a[:], w1_d[slot, :, :KH * D_FF])
                ib = nc.sync.dma_start(w1b[:], w1_d[slot, :, KH * D_FF:])
                wh.setdefault(slot, {})["w1"] = (w1a, w1b)
                wh[slot]["w1insts"] = (ia, ib)

            def load_w2(slot):
                # both halves on the (otherwise idle) gpsimd queue, strictly
                # after this slot's w1 so w1 gets full DMA bandwidth first
                KH2 = KC2 // 2
                w2sb = wp.tile([128, KC2, D_MODEL], BF16, tag="w2")
                ia = nc.scalar.dma_start(w2sb[:, :KH2, :],
                                         w2_d[slot, :, :KH2 * D_MODEL])
                ib = nc.sync.dma_start(w2sb[:, KH2:, :],
                                         w2_d[slot, :, KH2 * D_MODEL:])
                pa, pb = wh[slot]["w1insts"]
                add_dep_helper(pa.ins, ia.ins, reason="w2a after w1a for BW priority")
                add_dep_helper(pb.ins, ib.ins, reason="w2b after w1b for BW priority")
                wh.setdefault(slot, {})["w2"] = w2sb

            state = {}  # tile idx -> (gqt, alpha, slot)

            def phase_a(tg):
                slot, t = divmod(tg, T_E)
                if tg == 0:
                    load_w1(0)
                    load_w2(0)
                w1a, w1b = wh[slot]["w1"]
                xisb = xip.tile([128, KC1, 128], BF16, tag="xi")
                nc.sync.dma_start(xisb[:], xit_d[tg])

                rp = sb.tile([128, D_FF], F32, tag="rp")
                m8 = scp.tile([128, NJ1], F32, tag="m8")
                for nj in range(NJ1):
                    p1 = ps1.tile([128, 512], F32, tag="p1")
                    for kc in range(KC1):
                        wsrc = w1a if kc < KH else w1b
                        nc.tensor.matmul(
                            p1[:], xisb[:, kc, :],
                            wsrc[:, kc % KH, nj * 512:(nj + 1) * 512],
                            start=(kc == 0), stop=(kc == KC1 - 1))
                    nc.vector.reduce_max(m8[:, nj:nj + 1], p1[:],
                                         mybir.AxisListType.X)
                    nc.vector.tensor_scalar_max(
                        rp[:, nj * 512:(nj + 1) * 512], p1[:], 0.0)

                if t == T_E - 1 and slot + 1 < EXP_PER_CORE:
                    load_w1(slot + 1)
                    load_w2(slot + 1)

                # per-row scale chain (tiny (128,1) DVE ops)
                mh = scp.tile([128, 1], F32, tag="mh")
                nc.vector.reduce_max(mh[:], m8[:], mybir.AxisListType.X)
                maxg = scp.tile([128, 1], F32, tag="maxg")
                nc.vector.tensor_scalar_max(mh[:], mh[:], 0.0)
                nc.vector.tensor_mul(maxg[:], mh[:], mh[:])
                nc.vector.tensor_scalar_max(maxg[:], maxg[:], 1e-20)
                inv = scp.tile([128, 1], F32, tag="inv")
                t0 = scp.tile([128, 1], F32, tag="t0")
                nc.vector.reciprocal(inv[:], maxg[:])
                for _ in range(2):  # Newton: inv *= (2 - maxg*inv)
                    nc.vector.tensor_mul(t0[:], maxg[:], inv[:])
                    nc.vector.tensor_scalar(
                        t0[:], t0[:], -1.0, 2.0,
                        op0=mybir.AluOpType.mult, op1=mybir.AluOpType.add)
                    nc.vector.tensor_mul(inv[:], inv[:], t0[:])
                r127 = scp.tile([128, 1], F32, tag="r127")
                nc.vector.tensor_scalar_mul(r127[:], inv[:], 127.0)
                alpha = scp.tile([128, 1], F32, tag="alpha")
                nc.vector.tensor_mul(alpha[:], cvec[:, tg:tg + 1], maxg[:])

                # G = rp^2 ; quant: rp = RNE(G*r127) + MAGIC ; gq = rp - MAGIC (bf16)
                nc.scalar.activation(rp[:], rp[:],
                                     mybir.ActivationFunctionType.Square)
                nc.scalar.activation(rp[:], rp[:],
                                     mybir.ActivationFunctionType.Copy,
                                     scale=r127[:], bias=MAGIC)
                gqb = gqp.tile([128, D_FF], BF16, tag="gqb")
                nc.vector.tensor_scalar_sub(gqb[:], rp[:], MAGIC)

                gqt = gqp.tile([128, KC2, 128], BF16, tag="gqt")
                nc.sync.dma_start_transpose(gqt[:], gqb[:])
                state[tg] = (gqt, alpha, slot)

            def phase_b(tg):
                gqt, alpha, slot = state.pop(tg)
                w2sb = wh[slot]["w2"]
                outsb = gqp.tile([128, D_MODEL], F32, tag="outsb")
                for nj in range(NJ2):
                    p2 = ps2.tile([128, 512], F32, tag="p2")
                    for kc in range(KC2):
                        nc.tensor.matmul(
                            p2[:], gqt[:, kc, :],
                            w2sb[:, kc, nj * 512:(nj + 1) * 512],
                            start=(kc == 0), stop=(kc == KC2 - 1))
                    nc.scalar.activation(
                        outsb[:, nj * 512:(nj + 1) * 512], p2[:],
                        mybir.ActivationFunctionType.Copy, scale=alpha[:])
                nc.scalar.dma_start(out_d[tg * 128:(tg + 1) * 128, :], outsb[:])

            # software pipeline: A(t+1) issued before B(t) so PE never stalls.
            # At the expert-slot boundary, drain B early so both B phases cover
            # the next slot's weight DMA; keep-warm dummies cover the tail gap.
            phase_a(0)
            for tg in range(1, NT):
                if tg == T_E:
                    phase_b(tg - 1)          # B(T_E-1) before A(T_E)
                    # keep PE warm while slot1 weights stream in
                    dps = ps1.tile([128, 512], F32, tag="p1")
                    w2p = wh[0]["w2"]
                    for _ in range(48):
                        nc.tensor.matmul(dps[:], w2p[:, 0, :128],
                                         w2p[:, 1, :512], start=True, stop=True)
                    phase_a(tg)
                else:
                    phase_a(tg)
                    phase_b(tg - 1)
            # keep PE warm while the last tile's quant/transpose chain drains
    # (placeholder replaced below)
            dummy_ps = ps1.tile([128, 512], F32, tag="p1")
            w2last = wh[EXP_PER_CORE - 1]["w2"]
            for _ in range(26):
                nc.tensor.matmul(dummy_ps[:], w2last[:, 0, :128],
                                 w2last[:, 1, :512], start=True, stop=True)
            phase_b(NT - 1)
    nc.compile()
    return nc


def _get_program(T_E):
    if T_E not in _prog_cache:
        _prog_cache[T_E] = _build_program(T_E)
    return _prog_cache[T_E]


def kernel(x, w_router, w1, w2):
    import jax
    import jax.numpy as jnp
    from concourse.bass_utils import run_bass_kernel_spmd

    B, T, D = x.shape
    E = w1.shape[0]
    N = B * T

    # Routing must bitwise-match the harness's reference run. The reference
    # cannot run on the axon/trn backend (argsort unsupported), so the harness
    # runs it on CPU -> compute routing with the exact same jnp ops on CPU.
    cpu = jax.devices("cpu")[0]
    with jax.default_device(cpu):
        router_logits = jnp.einsum('btd,de->bte', x, w_router)
        router_probs = jax.nn.softmax(router_logits, axis=-1)
        top_probs, top_idx = jax.lax.top_k(router_probs, 1)
        top_probs = top_probs / (jnp.sum(top_probs, axis=-1, keepdims=True) + 1e-8)
        f = jnp.sum(jax.nn.one_hot(top_idx, E, dtype=x.dtype), axis=(0, 1, 2)) / (N * 1)
        p = jnp.mean(router_probs, axis=(0, 1))
        aux_loss = E * jnp.sum(f * p)

        ti = np.asarray(top_idx).reshape(-1)
        tp = np.asarray(top_probs).reshape(-1).astype(np.float32)

        # host quantization (same formulas as reference act/weight_quant)
        x_flat = jnp.reshape(x, (-1, D))
        s_tok = 127.0 / jnp.clip(jnp.max(jnp.abs(x_flat), axis=-1, keepdims=True), 1e-5, None)
        xi = jnp.clip(jnp.round(x_flat * s_tok), -128.0, 127.0)
        scale1 = jnp.clip(jnp.mean(jnp.abs(w1), axis=(1, 2), keepdims=True), 1e-5, None)
        w1t = jnp.clip(jnp.round(w1 / scale1), -1.0, 1.0)
        scale2 = jnp.clip(jnp.mean(jnp.abs(w2), axis=(1, 2), keepdims=True), 1e-5, None)
        w2t = jnp.clip(jnp.round(w2 / scale2), -1.0, 1.0)

    xi = np.asarray(xi, np.float32)
    s_tok = np.asarray(s_tok, np.float32).reshape(-1)
    scale1 = np.asarray(scale1, np.float32).reshape(-1)
    scale2 = np.asarray(scale2, np.float32).reshape(-1)
    w1t_bf = np.asarray(w1t, np.float32).astype(ml_dtypes.bfloat16)
    w2t_bf = np.asarray(w2t, np.float32).astype(ml_dtypes.bfloat16)

    # ---- dispatch bookkeeping ----
    order = np.argsort(ti, kind='stable')
    counts = np.bincount(ti, minlength=E)
    T_E = max(5, int(-(-counts.max() // 128)))
    CAP = T_E * 128
    NROWS = EXP_PER_CORE * CAP
    NT = EXP_PER_CORE * T_E
    offsets = np.concatenate([[0], np.cumsum(counts)[:-1]])

    # per-row combined scale: c = tp * scale2_e * (scale1_e / s_tok)^2 / 127
    KC1 = D // 128
    in_maps = []
    row_token = np.full((N_CORES, NROWS), -1, np.int64)
    for c in range(N_CORES):
        xrows = np.zeros((NROWS, D), np.float32)
        crows = np.zeros((NROWS,), np.float32)
        for slot in range(EXP_PER_CORE):
            e = c * EXP_PER_CORE + slot
            cnt = int(counts[e])
            toks = order[offsets[e]:offsets[e] + cnt]
            r0 = slot * CAP
            xrows[r0:r0 + cnt] = xi[toks]
            a1 = scale1[e] / s_tok[toks]
            crows[r0:r0 + cnt] = tp[toks] * scale2[e] * a1 * a1 / 127.0
            row_token[c, r0:r0 + cnt] = toks
        # per tile: (128 tok, D) -> (KC1, 128p, 128tok) -> (128p, KC1*128)
        xT = xrows.T.astype(ml_dtypes.bfloat16)          # (D, NROWS)
        xit = np.ascontiguousarray(
            xT.reshape(KC1, 128, NT, 128).transpose(2, 1, 0, 3)
        ).reshape(NT, 128, KC1 * 128)
        cvec = np.ascontiguousarray(crows.reshape(NT, 128).T)
        e0, e1 = c * EXP_PER_CORE, c * EXP_PER_CORE + 1
        def pmaj(w, kc):  # (rows, cols) -> (128, kc*cols) partition-major
            r, co = w.shape
            return np.ascontiguousarray(
                w.reshape(kc, 128, co).transpose(1, 0, 2)).reshape(128, kc * co)
        w1c = np.stack([pmaj(w1t_bf[e0], KC1), pmaj(w1t_bf[e1], KC1)])
        w2c = np.stack([pmaj(w2t_bf[e0], D_FF // 128), pmaj(w2t_bf[e1], D_FF // 128)])
        in_maps.append({"xit": xit, "w1c": w1c, "w2c": w2c, "cvec": cvec})

    nc = _get_program(T_E)
    res = run_bass_kernel_spmd(nc, in_maps, core_ids=list(range(N_CORES)))

    out_flat = np.zeros((N, D), np.float32)
    for c in range(N_CORES):
        oc = res.results[c]["outc"]
        mask = row_token[c] >= 0
        out_flat[row_token[c][mask]] = oc[mask]

    output = out_flat.reshape(B, T, D)
    return output, np.float32(aux_loss)


# revision 14
# speedup vs baseline: 1.1542x; 1.1542x over previous

# MoE BitNet FFN (E=16, D=1024, F=4096, top-1) on 8 Trainium2 NeuronCores.
#
# Strategy: expert-parallel, 2 experts per core. Routing/top-k/dispatch
# bookkeeping runs on host with the exact same jnp ops as the reference
# (bitwise-matching argmax decisions). The heavy FFN runs on device as
# bf16 matmuls over integer-valued quantized operands, which is EXACT:
#   xi  = clip(round(x*s), -128, 127)            ints in [-127,127] (bf16-exact)
#   w1t = clip(round(w1/scale1), -1, 1)          ternary (bf16-exact)
#   Hint = xi @ w1t                              ints |.| <= 2^17  (fp32 PSUM exact)
#   G    = relu(Hint)^2 ; gq = RNE(G*127/maxG)   ints in [0,127]
#   Oint = gq @ w2t                              ints |.| <= 2^19  (exact)
#   out  = Oint * (tp * scale2 * (scale1/s)^2 / 127) * maxG
# All data-dependent scale factors except maxG are host-precomputed per row.

import sys
for p in ("/opt/trn_rl_repo", "/root/.axon_site/_ro/trn_rl_repo"):
    if p not in sys.path:
        sys.path.append(p)

import numpy as np
import ml_dtypes

D_MODEL = 1024
D_FF = 4096
N_EXPERTS = 16
N_CORES = 8
EXP_PER_CORE = 2
MAGIC = 12582912.0  # 1.5 * 2^23, fp32 RNE magic
F32d = None  # set lazily
_prog_cache = {}


def _build_program(T_E):
    import concourse.mybir as mybir
    import concourse.tile as tile
    from concourse.tile_rust import add_dep_helper
    from concourse import bacc

    F32 = mybir.dt.float32
    BF16 = mybir.dt.bfloat16
    CAP = T_E * 128
    NROWS = EXP_PER_CORE * CAP
    NT = EXP_PER_CORE * T_E          # token tiles per core
    KC1 = D_MODEL // 128             # 8  k-chunks layer1
    NJ1 = D_FF // 512                # 8  n-chunks layer1
    KC2 = D_FF // 128                # 32 k-chunks layer2
    NJ2 = D_MODEL // 512             # 2  n-chunks layer2

    nc = bacc.Bacc(None, target_bir_lowering=False)
    xit_d = nc.dram_tensor("xit", (NT, 128, KC1 * 128), BF16, kind="ExternalInput")
    w1_d = nc.dram_tensor("w1c", (EXP_PER_CORE, 128, KC1 * D_FF), BF16, kind="ExternalInput")
    w2_d = nc.dram_tensor("w2c", (EXP_PER_CORE, 128, KC2 * D_MODEL), BF16, kind="ExternalInput")
    cv_d = nc.dram_tensor("cvec", (128, NT), F32, kind="ExternalInput")
    out_d = nc.dram_tensor("outc", (NROWS, D_MODEL), F32, kind="ExternalOutput")

    with tile.TileContext(nc) as tc:
        with (
            tc.tile_pool(name="wts", bufs=1) as wp,
            tc.tile_pool(name="sb", bufs=2) as sb,
            tc.tile_pool(name="xip", bufs=2) as xip,
            tc.tile_pool(name="gqp", bufs=1) as gqp,
            tc.tile_pool(name="scal", bufs=2) as scp,
            tc.tile_pool(name="ps1", bufs=3, space="PSUM") as ps1,
            tc.tile_pool(name="ps2", bufs=2, space="PSUM") as ps2,
        ):
            cvec = scp.tile([128, NT], F32, tag="cvec", bufs=1)
            nc.scalar.dma_start(cvec[:], cv_d[:])

            wh = {}  # slot -> {"w1": (w1a, w1b), "w2": tile}

            KH = KC1 // 2

            def load_w1(slot):
                # halves stream in parallel on two otherwise-idle DMA queues
                w1a = wp.tile([128, KH, D_FF], BF16, tag="w1a")
                w1b = wp.tile([128, KH, D_FF], BF16, tag="w1b")
                ia = nc.scalar.dma_start(w1a[:], w1_d[slot, :, :KH * D_FF])
                ib = nc.sync.dma_start(w1b[:], w1_d[slot, :, KH * D_FF:])
                wh.setdefault(slot, {})["w1"] = (w1a, w1b)
                wh[slot]["w1insts"] = (ia, ib)

            def load_w2(slot):
                # both halves on the (otherwise idle) gpsimd queue, strictly
                # after this slot's w1 so w1 gets full DMA bandwidth first
                KH2 = KC2 // 2
                w2sb = wp.tile([128, KC2, D_MODEL], BF16, tag="w2")
                ia = nc.gpsimd.dma_start(w2sb[:, :KH2, :],
                                          w2_d[slot, :, :KH2 * D_MODEL])
                ib = nc.gpsimd.dma_start(w2sb[:, KH2:, :],
                                          w2_d[slot, :, KH2 * D_MODEL:])
                pa, pb = wh[slot]["w1insts"]
                add_dep_helper(pa.ins, ia.ins, reason="w2a after w1a for BW priority")
                add_dep_helper(pb.ins, ib.ins, reason="w2b after w1b for BW priority")
                wh.setdefault(slot, {})["w2"] = w2sb

            state = {}  # tile idx -> (gqt, alpha, slot)

            def phase_a(tg):
                slot, t = divmod(tg, T_E)
                if tg == 0:
                    load_w1(0)
                    load_w2(0)
                w1a, w1b = wh[slot]["w1"]
                xisb = xip.tile([128, KC1, 128], BF16, tag="xi")
                nc.sync.dma_start(xisb[:], xit_d[tg])

                rp = sb.tile([128, D_FF], F32, tag="rp")
                m8 = scp.tile([128, NJ1], F32, tag="m8")
                for nj in range(NJ1):
                    p1 = ps1.tile([128, 512], F32, tag="p1")
                    for kc in range(KC1):
                        wsrc = w1a if kc < KH else w1b
                        nc.tensor.matmul(
                            p1[:], xisb[:, kc, :],
                            wsrc[:, kc % KH, nj * 512:(nj + 1) * 512],
                            start=(kc == 0), stop=(kc == KC1 - 1))
                    nc.vector.reduce_max(m8[:, nj:nj + 1], p1[:],
                                         mybir.AxisListType.X)
                    nc.vector.tensor_scalar_max(
                        rp[:, nj * 512:(nj + 1) * 512], p1[:], 0.0)

                if t == T_E - 1 and slot + 1 < EXP_PER_CORE:
                    load_w1(slot + 1)
                    load_w2(slot + 1)

                # per-row scale chain (tiny (128,1) DVE ops)
                mh = scp.tile([128, 1], F32, tag="mh")
                nc.vector.reduce_max(mh[:], m8[:], mybir.AxisListType.X)
                maxg = scp.tile([128, 1], F32, tag="maxg")
                nc.vector.tensor_scalar_max(mh[:], mh[:], 0.0)
                nc.vector.tensor_mul(maxg[:], mh[:], mh[:])
                nc.vector.tensor_scalar_max(maxg[:], maxg[:], 1e-20)
                inv = scp.tile([128, 1], F32, tag="inv")
                t0 = scp.tile([128, 1], F32, tag="t0")
                nc.vector.reciprocal(inv[:], maxg[:])
                for _ in range(2):  # Newton: inv *= (2 - maxg*inv)
                    nc.vector.tensor_mul(t0[:], maxg[:], inv[:])
                    nc.vector.tensor_scalar(
                        t0[:], t0[:], -1.0, 2.0,
                        op0=mybir.AluOpType.mult, op1=mybir.AluOpType.add)
                    nc.vector.tensor_mul(inv[:], inv[:], t0[:])
                r127 = scp.tile([128, 1], F32, tag="r127")
                nc.vector.tensor_scalar_mul(r127[:], inv[:], 127.0)
                alpha = scp.tile([128, 1], F32, tag="alpha")
                nc.vector.tensor_mul(alpha[:], cvec[:, tg:tg + 1], maxg[:])

                # G = rp^2 ; quant: rp = RNE(G*r127) + MAGIC ; gq = rp - MAGIC (bf16)
                nc.scalar.activation(rp[:], rp[:],
                                     mybir.ActivationFunctionType.Square)
                nc.scalar.activation(rp[:], rp[:],
                                     mybir.ActivationFunctionType.Copy,
                                     scale=r127[:], bias=MAGIC)
                gqb = gqp.tile([128, D_FF], BF16, tag="gqb")
                nc.vector.tensor_scalar_sub(gqb[:], rp[:], MAGIC)

                gqt = gqp.tile([128, KC2, 128], BF16, tag="gqt")
                nc.sync.dma_start_transpose(gqt[:], gqb[:])
                state[tg] = (gqt, alpha, slot)

            def phase_b(tg):
                gqt, alpha, slot = state.pop(tg)
                w2sb = wh[slot]["w2"]
                outsb = gqp.tile([128, D_MODEL], F32, tag="outsb")
                for nj in range(NJ2):
                    p2 = ps2.tile([128, 512], F32, tag="p2")
                    for kc in range(KC2):
                        nc.tensor.matmul(
                            p2[:], gqt[:, kc, :],
                            w2sb[:, kc, nj * 512:(nj + 1) * 512],
                            start=(kc == 0), stop=(kc == KC2 - 1))
                    nc.scalar.activation(
                        outsb[:, nj * 512:(nj + 1) * 512], p2[:],
                        mybir.ActivationFunctionType.Copy, scale=alpha[:])
                nc.scalar.dma_start(out_d[tg * 128:(tg + 1) * 128, :], outsb[:])

            # software pipeline: A(t+1) issued before B(t) so PE never stalls.
            # At the expert-slot boundary, drain B early so both B phases cover
            # the next slot's weight DMA; keep-warm dummies cover the tail gap.
            phase_a(0)
            for tg in range(1, NT):
                if tg == T_E:
                    phase_b(tg - 1)          # B(T_E-1) before A(T_E)
                    # keep PE warm while slot1 weights stream in
                    dps = ps1.tile([128, 512], F32, tag="p1")
                    w2p = wh[0]["w2"]
                    for _ in range(48):
                        nc.tensor.matmul(dps[:], w2p[:, 0, :128],
                                         w2p[:, 1, :512], start=True, stop=True)
                    phase_a(tg)
                else:
                    phase_a(tg)
                    phase_b(tg - 1)
            # keep PE warm while the last tile's quant/transpose chain drains
    # (placeholder replaced below)
            dummy_ps = ps1.tile([128, 512], F32, tag="p1")
            w2last = wh[EXP_PER_CORE - 1]["w2"]
            for _ in range(26):
                nc.tensor.matmul(dummy_ps[:], w2last[:, 0, :128],
                                 w2last[:, 1, :512], start=True, stop=True)
            phase_b(NT - 1)
    nc.compile()
    return nc


def _get_program(T_E):
    if T_E not in _prog_cache:
        _prog_cache[T_E] = _build_program(T_E)
    return _prog_cache[T_E]


def kernel(x, w_router, w1, w2):
    import jax
    import jax.numpy as jnp
    from concourse.bass_utils import run_bass_kernel_spmd

    B, T, D = x.shape
    E = w1.shape[0]
    N = B * T

    # Routing must bitwise-match the harness's reference run. The reference
    # cannot run on the axon/trn backend (argsort unsupported), so the harness
    # runs it on CPU -> compute routing with the exact same jnp ops on CPU.
    cpu = jax.devices("cpu")[0]
    with jax.default_device(cpu):
        router_logits = jnp.einsum('btd,de->bte', x, w_router)
        router_probs = jax.nn.softmax(router_logits, axis=-1)
        top_probs, top_idx = jax.lax.top_k(router_probs, 1)
        top_probs = top_probs / (jnp.sum(top_probs, axis=-1, keepdims=True) + 1e-8)
        f = jnp.sum(jax.nn.one_hot(top_idx, E, dtype=x.dtype), axis=(0, 1, 2)) / (N * 1)
        p = jnp.mean(router_probs, axis=(0, 1))
        aux_loss = E * jnp.sum(f * p)

        ti = np.asarray(top_idx).reshape(-1)
        tp = np.asarray(top_probs).reshape(-1).astype(np.float32)

        # host quantization (same formulas as reference act/weight_quant)
        x_flat = jnp.reshape(x, (-1, D))
        s_tok = 127.0 / jnp.clip(jnp.max(jnp.abs(x_flat), axis=-1, keepdims=True), 1e-5, None)
        xi = jnp.clip(jnp.round(x_flat * s_tok), -128.0, 127.0)
        scale1 = jnp.clip(jnp.mean(jnp.abs(w1), axis=(1, 2), keepdims=True), 1e-5, None)
        w1t = jnp.clip(jnp.round(w1 / scale1), -1.0, 1.0)
        scale2 = jnp.clip(jnp.mean(jnp.abs(w2), axis=(1, 2), keepdims=True), 1e-5, None)
        w2t = jnp.clip(jnp.round(w2 / scale2), -1.0, 1.0)

    xi = np.asarray(xi, np.float32)
    s_tok = np.asarray(s_tok, np.float32).reshape(-1)
    scale1 = np.asarray(scale1, np.float32).reshape(-1)
    scale2 = np.asarray(scale2, np.float32).reshape(-1)
    w1t_bf = np.asarray(w1t, np.float32).astype(ml_dtypes.bfloat16)
    w2t_bf = np.asarray(w2t, np.float32).astype(ml_dtypes.bfloat16)

    # ---- dispatch bookkeeping ----
    order = np.argsort(ti, kind='stable')
    counts = np.bincount(ti, minlength=E)
    T_E = max(5, int(-(-counts.max() // 128)))
    CAP = T_E * 128
    NROWS = EXP_PER_CORE * CAP
    NT = EXP_PER_CORE * T_E
    offsets = np.concatenate([[0], np.cumsum(counts)[:-1]])

    # per-row combined scale: c = tp * scale2_e * (scale1_e / s_tok)^2 / 127
    KC1 = D // 128
    in_maps = []
    row_token = np.full((N_CORES, NROWS), -1, np.int64)
    for c in range(N_CORES):
        xrows = np.zeros((NROWS, D), np.float32)
        crows = np.zeros((NROWS,), np.float32)
        for slot in range(EXP_PER_CORE):
            e = c * EXP_PER_CORE + slot
            cnt = int(counts[e])
            toks = order[offsets[e]:offsets[e] + cnt]
            r0 = slot * CAP
            xrows[r0:r0 + cnt] = xi[toks]
            a1 = scale1[e] / s_tok[toks]
            crows[r0:r0 + cnt] = tp[toks] * scale2[e] * a1 * a1 / 127.0
            row_token[c, r0:r0 + cnt] = toks
        # per tile: (128 tok, D) -> (KC1, 128p, 128tok) -> (128p, KC1*128)
        xT = xrows.T.astype(ml_dtypes.bfloat16)          # (D, NROWS)
        xit = np.ascontiguousarray(
            xT.reshape(KC1, 128, NT, 128).transpose(2, 1, 0, 3)
        ).reshape(NT, 128, KC1 * 128)
        cvec = np.ascontiguousarray(crows.reshape(NT, 128).T)
        e0, e1 = c * EXP_PER_CORE, c * EXP_PER_CORE + 1
        def pmaj(w, kc):  # (rows, cols) -> (128, kc*cols) partition-major
            r, co = w.shape
            return np.ascontiguousarray(
                w.reshape(kc, 128, co).transpose(1, 0, 2)).reshape(128, kc * co)
        w1c = np.stack([pmaj(w1t_bf[e0], KC1), pmaj(w1t_bf[e1], KC1)])
        w2c = np.stack([pmaj(w2t_bf[e0], D_FF // 128), pmaj(w2t_bf[e1], D_FF // 128)])
        in_maps.append({"xit": xit, "w1c": w1c, "w2c": w2c, "cvec": cvec})

    nc = _get_program(T_E)
    res = run_bass_kernel_spmd(nc, in_maps, core_ids=list(range(N_CORES)))

    out_flat = np.zeros((N, D), np.float32)
    for c in range(N_CORES):
        oc = res.results[c]["outc"]
        mask = row_token[c] >= 0
        out_flat[row_token[c][mask]] = oc[mask]

    output = out_flat.reshape(B, T, D)
    return output, np.float32(aux_loss)


# revision 15
# speedup vs baseline: 1.3668x; 1.1841x over previous

# MoE BitNet FFN (E=16, D=1024, F=4096, top-1) on 8 Trainium2 NeuronCores.
#
# Strategy: expert-parallel, 2 experts per core. Routing/top-k/dispatch
# bookkeeping runs on host with the exact same jnp ops as the reference
# (bitwise-matching argmax decisions). The heavy FFN runs on device as
# bf16 matmuls over integer-valued quantized operands, which is EXACT:
#   xi  = clip(round(x*s), -128, 127)            ints in [-127,127] (bf16-exact)
#   w1t = clip(round(w1/scale1), -1, 1)          ternary (bf16-exact)
#   Hint = xi @ w1t                              ints |.| <= 2^17  (fp32 PSUM exact)
#   G    = relu(Hint)^2 ; gq = RNE(G*127/maxG)   ints in [0,127]
#   Oint = gq @ w2t                              ints |.| <= 2^19  (exact)
#   out  = Oint * (tp * scale2 * (scale1/s)^2 / 127) * maxG
# All data-dependent scale factors except maxG are host-precomputed per row.

import sys
for p in ("/opt/trn_rl_repo", "/root/.axon_site/_ro/trn_rl_repo"):
    if p not in sys.path:
        sys.path.append(p)

import numpy as np
import ml_dtypes

D_MODEL = 1024
D_FF = 4096
N_EXPERTS = 16
N_CORES = 8
EXP_PER_CORE = 2
MAGIC = 12582912.0  # 1.5 * 2^23, fp32 RNE magic
F32d = None  # set lazily
_prog_cache = {}


def _build_program(T_E):
    import concourse.mybir as mybir
    import concourse.tile as tile
    from concourse.tile_rust import add_dep_helper
    from concourse import bacc

    F32 = mybir.dt.float32
    BF16 = mybir.dt.bfloat16
    CAP = T_E * 128
    NROWS = EXP_PER_CORE * CAP
    NT = EXP_PER_CORE * T_E          # token tiles per core
    KC1 = D_MODEL // 128             # 8  k-chunks layer1
    NJ1 = D_FF // 512                # 8  n-chunks layer1
    KC2 = D_FF // 128                # 32 k-chunks layer2
    NJ2 = D_MODEL // 512             # 2  n-chunks layer2

    nc = bacc.Bacc(None, target_bir_lowering=False)
    xit_d = nc.dram_tensor("xit", (NT, 128, KC1 * 128), BF16, kind="ExternalInput")
    w1_d = nc.dram_tensor("w1c", (EXP_PER_CORE, 128, KC1 * D_FF), BF16, kind="ExternalInput")
    w2_d = nc.dram_tensor("w2c", (EXP_PER_CORE, 128, KC2 * D_MODEL), BF16, kind="ExternalInput")
    cv_d = nc.dram_tensor("cvec", (128, NT), F32, kind="ExternalInput")
    out_d = nc.dram_tensor("outc", (NROWS, D_MODEL), F32, kind="ExternalOutput")

    with tile.TileContext(nc) as tc:
        with (
            tc.tile_pool(name="wts", bufs=1) as wp,
            tc.tile_pool(name="sb", bufs=2) as sb,
            tc.tile_pool(name="xip", bufs=2) as xip,
            tc.tile_pool(name="gqp", bufs=1) as gqp,
            tc.tile_pool(name="scal", bufs=2) as scp,
            tc.tile_pool(name="ps1", bufs=3, space="PSUM") as ps1,
            tc.tile_pool(name="ps2", bufs=2, space="PSUM") as ps2,
        ):
            KH = KC1 // 2
            KH2 = KC2 // 2
            cvec = scp.tile([128, NT], F32, tag="cvec", bufs=1)
            nc.scalar.dma_start(cvec[:], cv_d[:])

            wh = {}  # slot -> {"w1": (w1a, w1b), "w2": tile}

            def load_w1(slot):
                # halves stream in parallel on the two HWDGE queues (FIFO rings)
                w1a = wp.tile([128, KH, D_FF], BF16, tag="w1a")
                w1b = wp.tile([128, KH, D_FF], BF16, tag="w1b")
                nc.scalar.dma_start(w1a[:], w1_d[slot, :, :KH * D_FF])
                nc.sync.dma_start(w1b[:], w1_d[slot, :, KH * D_FF:])
                wh.setdefault(slot, {})["w1"] = (w1a, w1b)

            def load_w2a(slot):
                # first half right behind w1a in the scalar ring
                w2sb = wp.tile([128, KC2, D_MODEL], BF16, tag="w2")
                nc.scalar.dma_start(w2sb[:, :KH2, :], w2_d[slot, :, :KH2 * D_MODEL])
                wh.setdefault(slot, {})["w2"] = w2sb

            def load_w2b(slot):
                # second half in the sync ring (after the current tile's xi)
                w2sb = wh[slot]["w2"]
                nc.sync.dma_start(w2sb[:, KH2:, :], w2_d[slot, :, KH2 * D_MODEL:])

            state = {}  # tile idx -> (gqt, alpha, slot)

            def phase_a(tg):
                slot, t = divmod(tg, T_E)
                if tg == 0:
                    load_w1(0)
                    load_w2a(0)
                w1a, w1b = wh[slot]["w1"]
                xisb = xip.tile([128, KC1, 128], BF16, tag="xi")
                nc.sync.dma_start(xisb[:], xit_d[tg])
                if t == 0:
                    load_w2b(slot)

                rp = sb.tile([128, D_FF], F32, tag="rp")
                m8 = scp.tile([128, NJ1], F32, tag="m8")
                for nj in range(NJ1):
                    p1 = ps1.tile([128, 512], F32, tag="p1")
                    for kc in range(KC1):
                        wsrc = w1a if kc < KH else w1b
                        nc.tensor.matmul(
                            p1[:], xisb[:, kc, :],
                            wsrc[:, kc % KH, nj * 512:(nj + 1) * 512],
                            start=(kc == 0), stop=(kc == KC1 - 1))
                    nc.vector.reduce_max(m8[:, nj:nj + 1], p1[:],
                                         mybir.AxisListType.X)
                    nc.vector.tensor_scalar_max(
                        rp[:, nj * 512:(nj + 1) * 512], p1[:], 0.0)

                # prefetch next slot's weights right after this slot's last
                # w1 use; FIFO ring order gives w1' priority over w2a'
                if t == T_E - 1 and slot + 1 < EXP_PER_CORE:
                    load_w1(slot + 1)
                    load_w2a(slot + 1)

                # per-row scale chain (tiny (128,1) DVE ops)
                mh = scp.tile([128, 1], F32, tag="mh")
                nc.vector.reduce_max(mh[:], m8[:], mybir.AxisListType.X)
                maxg = scp.tile([128, 1], F32, tag="maxg")
                nc.vector.tensor_scalar_max(mh[:], mh[:], 0.0)
                nc.vector.tensor_mul(maxg[:], mh[:], mh[:])
                nc.vector.tensor_scalar_max(maxg[:], maxg[:], 1e-20)
                inv = scp.tile([128, 1], F32, tag="inv")
                t0 = scp.tile([128, 1], F32, tag="t0")
                nc.vector.reciprocal(inv[:], maxg[:])
                for _ in range(2):  # Newton: inv *= (2 - maxg*inv)
                    nc.vector.tensor_mul(t0[:], maxg[:], inv[:])
                    nc.vector.tensor_scalar(
                        t0[:], t0[:], -1.0, 2.0,
                        op0=mybir.AluOpType.mult, op1=mybir.AluOpType.add)
                    nc.vector.tensor_mul(inv[:], inv[:], t0[:])
                r127 = scp.tile([128, 1], F32, tag="r127")
                nc.vector.tensor_scalar_mul(r127[:], inv[:], 127.0)
                alpha = scp.tile([128, 1], F32, tag="alpha")
                nc.vector.tensor_mul(alpha[:], cvec[:, tg:tg + 1], maxg[:])

                # G = rp^2 ; quant: rp = RNE(G*r127) + MAGIC ; gq = rp - MAGIC
                nc.scalar.activation(rp[:], rp[:],
                                     mybir.ActivationFunctionType.Square)
                nc.scalar.activation(rp[:], rp[:],
                                     mybir.ActivationFunctionType.Copy,
                                     scale=r127[:], bias=MAGIC)
                gqb = gqp.tile([128, D_FF], BF16, tag="gqb")
                nc.vector.tensor_scalar_sub(gqb[:], rp[:], MAGIC)

                gqt = gqp.tile([128, KC2, 128], BF16, tag="gqt")
                nc.sync.dma_start_transpose(gqt[:], gqb[:])
                state[tg] = (gqt, alpha, slot)

            def phase_b(tg):
                gqt, alpha, slot = state.pop(tg)
                w2sb = wh[slot]["w2"]
                outsb = gqp.tile([128, D_MODEL], F32, tag="outsb")
                for nj in range(NJ2):
                    p2 = ps2.tile([128, 512], F32, tag="p2")
                    for kc in range(KC2):
                        nc.tensor.matmul(
                            p2[:], gqt[:, kc, :],
                            w2sb[:, kc, nj * 512:(nj + 1) * 512],
                            start=(kc == 0), stop=(kc == KC2 - 1))
                    nc.scalar.activation(
                        outsb[:, nj * 512:(nj + 1) * 512], p2[:],
                        mybir.ActivationFunctionType.Copy, scale=alpha[:])
                nc.scalar.dma_start(out_d[tg * 128:(tg + 1) * 128, :], outsb[:])

            # software pipeline: A(t+1) before B(t) so PE never stalls
            phase_a(0)
            for tg in range(1, NT):
                if tg == T_E:
                    phase_b(tg - 1)      # B(T_E-1) before A(T_E)
                    phase_a(tg)
                else:
                    phase_a(tg)
                    phase_b(tg - 1)
            # keep PE warm while the last tile's quant/transpose chain drains
            dummy_ps = ps1.tile([128, 512], F32, tag="p1")
            w2last = wh[EXP_PER_CORE - 1]["w2"]
            for _ in range(26):
                nc.tensor.matmul(dummy_ps[:], w2last[:, 0, :128],
                                 w2last[:, 1, :512], start=True, stop=True)
            phase_b(NT - 1)
    nc.compile()
    return nc


def _get_program(T_E):
    if T_E not in _prog_cache:
        _prog_cache[T_E] = _build_program(T_E)
    return _prog_cache[T_E]


def kernel(x, w_router, w1, w2):
    import jax
    import jax.numpy as jnp
    from concourse.bass_utils import run_bass_kernel_spmd

    B, T, D = x.shape
    E = w1.shape[0]
    N = B * T

    # Routing must bitwise-match the harness's reference run. The reference
    # cannot run on the axon/trn backend (argsort unsupported), so the harness
    # runs it on CPU -> compute routing with the exact same jnp ops on CPU.
    cpu = jax.devices("cpu")[0]
    with jax.default_device(cpu):
        router_logits = jnp.einsum('btd,de->bte', x, w_router)
        router_probs = jax.nn.softmax(router_logits, axis=-1)
        top_probs, top_idx = jax.lax.top_k(router_probs, 1)
        top_probs = top_probs / (jnp.sum(top_probs, axis=-1, keepdims=True) + 1e-8)
        f = jnp.sum(jax.nn.one_hot(top_idx, E, dtype=x.dtype), axis=(0, 1, 2)) / (N * 1)
        p = jnp.mean(router_probs, axis=(0, 1))
        aux_loss = E * jnp.sum(f * p)

        ti = np.asarray(top_idx).reshape(-1)
        tp = np.asarray(top_probs).reshape(-1).astype(np.float32)

        # host quantization (same formulas as reference act/weight_quant)
        x_flat = jnp.reshape(x, (-1, D))
        s_tok = 127.0 / jnp.clip(jnp.max(jnp.abs(x_flat), axis=-1, keepdims=True), 1e-5, None)
        xi = jnp.clip(jnp.round(x_flat * s_tok), -128.0, 127.0)
        scale1 = jnp.clip(jnp.mean(jnp.abs(w1), axis=(1, 2), keepdims=True), 1e-5, None)
        w1t = jnp.clip(jnp.round(w1 / scale1), -1.0, 1.0)
        scale2 = jnp.clip(jnp.mean(jnp.abs(w2), axis=(1, 2), keepdims=True), 1e-5, None)
        w2t = jnp.clip(jnp.round(w2 / scale2), -1.0, 1.0)

    xi = np.asarray(xi, np.float32)
    s_tok = np.asarray(s_tok, np.float32).reshape(-1)
    scale1 = np.asarray(scale1, np.float32).reshape(-1)
    scale2 = np.asarray(scale2, np.float32).reshape(-1)
    w1t_bf = np.asarray(w1t, np.float32).astype(ml_dtypes.bfloat16)
    w2t_bf = np.asarray(w2t, np.float32).astype(ml_dtypes.bfloat16)

    # ---- dispatch bookkeeping ----
    order = np.argsort(ti, kind='stable')
    counts = np.bincount(ti, minlength=E)
    T_E = max(5, int(-(-counts.max() // 128)))
    CAP = T_E * 128
    NROWS = EXP_PER_CORE * CAP
    NT = EXP_PER_CORE * T_E
    offsets = np.concatenate([[0], np.cumsum(counts)[:-1]])

    # per-row combined scale: c = tp * scale2_e * (scale1_e / s_tok)^2 / 127
    KC1 = D // 128
    in_maps = []
    row_token = np.full((N_CORES, NROWS), -1, np.int64)
    for c in range(N_CORES):
        xrows = np.zeros((NROWS, D), np.float32)
        crows = np.zeros((NROWS,), np.float32)
        for slot in range(EXP_PER_CORE):
            e = c * EXP_PER_CORE + slot
            cnt = int(counts[e])
            toks = order[offsets[e]:offsets[e] + cnt]
            r0 = slot * CAP
            xrows[r0:r0 + cnt] = xi[toks]
            a1 = scale1[e] / s_tok[toks]
            crows[r0:r0 + cnt] = tp[toks] * scale2[e] * a1 * a1 / 127.0
            row_token[c, r0:r0 + cnt] = toks
        # per tile: (128 tok, D) -> (KC1, 128p, 128tok) -> (128p, KC1*128)
        xT = xrows.T.astype(ml_dtypes.bfloat16)          # (D, NROWS)
        xit = np.ascontiguousarray(
            xT.reshape(KC1, 128, NT, 128).transpose(2, 1, 0, 3)
        ).reshape(NT, 128, KC1 * 128)
        cvec = np.ascontiguousarray(crows.reshape(NT, 128).T)
        e0, e1 = c * EXP_PER_CORE, c * EXP_PER_CORE + 1
        def pmaj(w, kc):  # (rows, cols) -> (128, kc*cols) partition-major
            r, co = w.shape
            return np.ascontiguousarray(
                w.reshape(kc, 128, co).transpose(1, 0, 2)).reshape(128, kc * co)
        w1c = np.stack([pmaj(w1t_bf[e0], KC1), pmaj(w1t_bf[e1], KC1)])
        w2c = np.stack([pmaj(w2t_bf[e0], D_FF // 128), pmaj(w2t_bf[e1], D_FF // 128)])
        in_maps.append({"xit": xit, "w1c": w1c, "w2c": w2c, "cvec": cvec})

    nc = _get_program(T_E)
    res = run_bass_kernel_spmd(nc, in_maps, core_ids=list(range(N_CORES)))

    out_flat = np.zeros((N, D), np.float32)
    for c in range(N_CORES):
        oc = res.results[c]["outc"]
        mask = row_token[c] >= 0
        out_flat[row_token[c][mask]] = oc[mask]

    output = out_flat.reshape(B, T, D)
    return output, np.float32(aux_loss)
